# revision 23
# baseline (speedup 1.0000x reference)
"""GQA decode attention (B=32, q_len=1, T=4096, 32 q heads / 8 kv heads, hd=128)
on 8 Trainium2 NeuronCores.

Sharding: tensor-parallel over kv heads - core h owns kv head h (4 q heads),
its slice of wq/wk/wv (ColumnParallel) and wo (RowParallel), and the
cache_k/cache_v slices for that head. Each core computes a partial output
(RowParallel wo) in transposed layout; the host sums the 8 partials.

The kernel is HBM-bandwidth-bound (KV cache streaming), so everything is
fp16 end to end (rel err ~6e-4 vs the fp32 reference, measured on the
actual data):
  - q_len==1 means RoPE is a fixed linear map on the projection outputs, so
    it is folded into wq/wk on the host: w_rot = R(freqs) @ w. The
    1/sqrt(head_dim) score scale is folded into wq too.
  - K cache is stored transposed [hd, t] in fp16: one score matmul per
    128-key tile (K-tile stationary, fast-weight-load path; q streams 4
    columns).
  - V cache is stored [t, d] in fp16 and used stationary in the PV matmul
    (probs stream 4 columns), producing attn directly in [d, g] layout -
    no per-batch transpose.
  - softmax runs unnormalized (exp in fp32 PSUM -> fp16 probs); the
    denominator comes from a ones-column matmul (column sums) + a strided
    DVE reduce, and the normalization uses a ones-matmul broadcast of
    1/sum across partitions.
  - big DMA is split over three hardware rings: K on the sync ring, V on
    the scalar ring, weights/consts on the gpsimd ring, so the 16 DMA
    engines see deeper queues.
"""

import numpy as np

B = 32
DIM = 4096
HD = 128
NKV = 8
NG = 4          # q heads per kv head
T = 4096
NT = 32         # T / 128 key tiles
ND = 32         # DIM / 128 contraction chunks
N_CORES = 8

_PROG_CACHE = {}


def _build_program():
    import concourse.mybir as mybir
    import concourse.tile as tile
    from concourse import bacc

    fp32 = mybir.dt.float32
    f16 = mybir.dt.float16
    af = mybir.ActivationFunctionType
    ax = mybir.AxisListType
    alu = mybir.AluOpType

    nc = bacc.Bacc("TRN2", target_bir_lowering=False, debug=False,
                   num_devices=N_CORES)

    xTp_d = nc.dram_tensor("xTp", [128, ND * B], f16, kind="ExternalInput").ap()
    # wqkvT relayout [p, (piece chunk col)]: 4 pieces of 8 dk-chunks, so each
    # piece is one DMA with 8*768*2 = 12KB contiguous per partition
    wqkvT_d = nc.dram_tensor("wqkvT", [128, 4 * 8 * 768], f16,
                             kind="ExternalInput").ap()
    # woT relayout [p, (g col)]: one DMA, 32KB contiguous per partition
    woT_d = nc.dram_tensor("woT", [128, NG * DIM], f16, kind="ExternalInput").ap()
    KT_d = nc.dram_tensor("KT", [B, HD, T], f16, kind="ExternalInput").ap()
    Vp_d = nc.dram_tensor("Vp", [B, 128, T], f16, kind="ExternalInput").ap()
    ones_d = nc.dram_tensor("ones", [128, 128], f16, kind="ExternalInput").ap()
    # transposed partial output, layout [p, (dchunk, b)]
    out_d = nc.dram_tensor("outT", [128, 32 * B], f16, kind="ExternalOutput").ap()

    with tile.TileContext(nc) as tc:
        from contextlib import ExitStack
        with ExitStack() as ctx:
            const_pool = ctx.enter_context(tc.tile_pool(name="const", bufs=1))
            wpool = ctx.enter_context(tc.tile_pool(name="w", bufs=2))
            kv_pool = ctx.enter_context(tc.tile_pool(name="kv", bufs=8))
            small = ctx.enter_context(tc.tile_pool(name="small", bufs=3))

            # consts + weights on the gpsimd ring (keeps sync/scalar rings
            # free for the KV stream)
            xTp_sb = const_pool.tile([128, ND * B], f16, name="xTp_sb")
            nc.sync.dma_start(xTp_sb[:], xTp_d[:])
            ones_sb = const_pool.tile([128, 128], f16, name="ones_sb")
            nc.scalar.dma_start(ones_sb[:], ones_d[:])
            # woT tile: loaded in a single big-packet DMA, delayed behind the
            # projections (see the dummy-dep below) so the startup bandwidth
            # all goes to the wqkv weights that gate attention compute
            woT_sb = const_pool.tile([128, NG * DIM], f16, name="woT_sb")

            # ---- QKV projections: qT[o,b], kT[o,b], v[b,o] ----
            qT_sb = const_pool.tile([128, NG * B], f16, name="qT_sb")
            kT_sb = const_pool.tile([128, B], f16, name="kT_sb")
            v_sb = const_pool.tile([B, HD], f16, name="v_sb")

            # wqkv pieces: big-packet DMAs on the two fast rings (the gpsimd
            # ring is ~2x slower per-queue)
            wp_engs = [nc.sync, nc.scalar, nc.sync, nc.scalar]

            with tc.tile_pool(name="ppsum", bufs=1, space="PSUM") as ppsum:
                psq = [ppsum.tile([128, B], fp32, name=f"psq{g}", tag=f"psq{g}")
                       for g in range(NG)]
                psk = ppsum.tile([128, B], fp32, name="psk", tag="psk")
                psv = ppsum.tile([B, HD], fp32, name="psv", tag="psv")
                for p in range(4):
                    wp = wpool.tile([128, 8 * 768], f16, name="wp", tag="wp")
                    wp_engs[p].dma_start(
                        wp[:], wqkvT_d[:, 8 * 768 * p:8 * 768 * (p + 1)])
                    for i in range(8):
                        n = 8 * p + i
                        wch = wp[:, 768 * i:768 * (i + 1)]
                        xch = xTp_sb[:, B * n:B * (n + 1)]
                        st, sp = (n == 0), (n == ND - 1)
                        for g in range(NG):
                            nc.tensor.matmul(psq[g][:],
                                             wch[:, 128 * g:128 * (g + 1)],
                                             xch, start=st, stop=sp)
                        nc.tensor.matmul(psk[:], wch[:, 512:640], xch,
                                         start=st, stop=sp)
                        nc.tensor.matmul(psv[:], xch, wch[:, 640:768],
                                         start=st, stop=sp)
                for g in range(NG):
                    nc.vector.tensor_copy(qT_sb[:, B * g:B * (g + 1)], psq[g][:])
                nc.vector.tensor_copy(kT_sb[:], psk[:])
                nc.vector.tensor_copy(v_sb[:], psv[:])

            spsum = ctx.enter_context(tc.tile_pool(name="spsum", bufs=3, space="PSUM"))
            opsum = ctx.enter_context(tc.tile_pool(name="opsum", bufs=3, space="PSUM"))
            wpsum = ctx.enter_context(tc.tile_pool(name="wpsum", bufs=2, space="PSUM"))

            qT_re = qT_sb.rearrange("p (g b) -> p b g", b=B)
            attnT_sb = const_pool.tile([128, NG * B], f16, name="attnT_sb")
            attnT_re = attnT_sb.rearrange("p (g b) -> p b g", b=B)

            outT_sb = const_pool.tile([128, 32 * B], f16, name="outT_sb")

            def wo_part(h0, nb):
                # out[dout, b] for batches [h0, h0+nb): woT-stationary
                for j in range(32):
                    psW = wpsum.tile([128, nb], fp32, name="psW", tag="psW")
                    for g in range(NG):
                        nc.tensor.matmul(
                            psW[:],
                            woT_sb[:, DIM * g + 128 * j:DIM * g + 128 * (j + 1)],
                            attnT_sb[:, B * g + h0:B * g + h0 + nb],
                            start=(g == 0), stop=(g == NG - 1))
                    nc.vector.tensor_copy(
                        outT_sb[:, B * j + h0:B * j + h0 + nb], psW[:])

            # ---- attention, software-pipelined (depth 2) ----
            # iteration i emits: loads+scores+exp for batch i, PV+colsum+
            # denominator chain for batch i-1, broadcast+normalize for batch
            # i-2 -- so every tensor-queue op has its deps resolved a full
            # batch ahead and the in-order tensor engine never stalls on the
            # DVE round-trip.
            Vt = {}     # per-batch live tiles
            Pt = {}
            Ot = {}
            RRt = {}

            def stage_load_scores(b):
                ek, ev = ((nc.sync, nc.scalar) if b % 2 == 0
                          else (nc.scalar, nc.sync))
                K_sb = kv_pool.tile([128, T], f16, name="K_sb", tag="K")
                ek.dma_start(K_sb[:], KT_d[b])
                V_sb = kv_pool.tile([128, T], f16, name="V_sb", tag="V")
                ev.dma_start(V_sb[:], Vp_d[b])
                Vt[b] = V_sb
                # new-token key: overwrite cache column t=4095
                nc.vector.tensor_copy(K_sb[:, T - 1:T], kT_sb[:, b:b + 1])
                # new-token value: overwrite the t=4095 V row (partition 127
                # of the last chunk). Cross-partition move -> tiny DMA.
                nc.gpsimd.dma_start(
                    V_sb[127:128, 128 * (NT - 1):128 * NT],
                    v_sb[b:b + 1, 0:HD])
                qb = qT_re[:, b]  # [128, 4] strided
                psS = spsum.tile([128, NG * NT], fp32, name="psS", tag="psS")
                for n in range(NT):
                    nc.tensor.matmul(psS[:, NG * n:NG * (n + 1)],
                                     K_sb[:, 128 * n:128 * (n + 1)], qb,
                                     start=True, stop=True)
                probs = kv_pool.tile([128, NG * NT], f16, name="probs",
                                     tag="probs")
                for c in range(2):
                    cw = NG * NT // 2
                    nc.scalar.activation(probs[:, cw * c:cw * (c + 1)],
                                         psS[:, cw * c:cw * (c + 1)], af.Exp)
                Pt[b] = probs

            def stage_pv_denom(b):
                V_sb, probs = Vt.pop(b), Pt[b]
                # one PSUM bank: cols [0,4) = PV out [d, g]; cols [4,8) =
                # broadcast 1/sum; cols [8,136) partition 0 = column sums
                psO = opsum.tile([128, 8 + NG * NT], fp32, name="psO",
                                 tag="psO")
                for n in range(NT):
                    nc.tensor.matmul(psO[:, 0:NG],
                                     V_sb[:, 128 * n:128 * (n + 1)],
                                     probs[:, NG * n:NG * (n + 1)],
                                     start=(n == 0), stop=(n == NT - 1))
                nc.tensor.matmul(psO[0:1, 8:8 + NG * NT], ones_sb[:, 0:1],
                                 probs[:], start=True, stop=True)
                Ot[b] = psO
                sums4 = small.tile([1, NG], fp32, name="sums4", tag="sums4")
                nc.vector.tensor_reduce(
                    sums4[:],
                    psO[0:1, 8:8 + NG * NT].rearrange("p (n g) -> p g n", g=NG),
                    axis=ax.X, op=alu.add)
                recip = small.tile([1, NG], fp32, name="recip", tag="recip")
                nc.vector.reciprocal(recip[:], sums4[:])
                rr = small.tile([128, NG], f16, name="rr", tag="rr")
                nc.vector.memset(rr[:], 0.0)
                nc.vector.tensor_copy(rr[0:1, :], recip[:])
                RRt[b] = rr

            def stage_normalize(b):
                psO, rr = Ot.pop(b), RRt.pop(b)
                del Pt[b]
                nc.tensor.matmul(psO[:, NG:2 * NG], ones_sb[:], rr[:],
                                 start=True, stop=True)
                bc_sb = small.tile([128, NG], fp32, name="bc_sb", tag="bc_sb")
                nc.vector.tensor_copy(bc_sb[:], psO[:, NG:2 * NG])
                nc.vector.tensor_mul(attnT_re[:, b], psO[:, 0:NG], bc_sb[:])

            for i in range(B + 2):
                if i == 20:
                    # woT: hold behind attnT[16] so the 4.2MB burst lands in
                    # the late-stream slack of the otherwise-idle gpsimd ring
                    nc.vector.tensor_copy(woT_sb[0:1, 0:1],
                                          attnT_sb[0:1, 16:17])
                    nc.gpsimd.dma_start(woT_sb[:], woT_d[:])
                if i == 28:
                    # wo for batches 0..15: tensor-idle-gap work while the
                    # last batches stream (needs woT + attnT[0:16])
                    wo_part(0, 16)
                if i >= 2:
                    stage_normalize(i - 2)
                if 1 <= i <= B:
                    stage_pv_denom(i - 1)
                if i < B:
                    stage_load_scores(i)

            # ---- second wo half (batches 16..31) + output store ----
            wo_part(B // 2, B // 2)
            nc.sync.dma_start(out_d[:], outT_sb[:])

    nc.compile()
    return nc


def _get_program():
    if "nc" not in _PROG_CACHE:
        _PROG_CACHE["nc"] = _build_program()
    return _PROG_CACHE["nc"]


def _host_prep(x, freqs_cos, freqs_sin, cache_k, cache_v, wq, wk, wv, wo):
    """Build the 8 per-core input maps (all fp16)."""
    f32 = np.float32
    f16 = np.float16
    x = np.asarray(x, f32)
    cos = np.asarray(freqs_cos, f32).reshape(-1)[:HD // 2]
    sin = np.asarray(freqs_sin, f32).reshape(-1)[:HD // 2]
    wq = np.asarray(wq, f32)
    wk = np.asarray(wk, f32)
    wv = np.asarray(wv, f32)
    wo = np.asarray(wo, f32)
    cache_k = np.asarray(cache_k, f32)
    cache_v = np.asarray(cache_v, f32)

    def rope_fold(w, nheads):
        w4 = w.reshape(nheads, HD // 2, 2, DIM)
        a, bb = w4[:, :, 0, :], w4[:, :, 1, :]
        c = cos[None, :, None]
        s = sin[None, :, None]
        out = np.empty_like(w4)
        out[:, :, 0, :] = a * c - bb * s
        out[:, :, 1, :] = a * s + bb * c
        return out.reshape(nheads * HD, DIM)

    wq_r = rope_fold(wq, NKV * NG) * f32(1.0 / np.sqrt(HD))
    wk_r = rope_fold(wk, NKV)

    x2 = x.reshape(B, DIM)
    xTp = np.ascontiguousarray(
        x2.T.reshape(ND, 128, B).transpose(1, 0, 2)).reshape(128, ND * B)
    xTp = xTp.astype(f16)

    # K transposed per (h, b): [h, b, d, t]
    KT_all = np.ascontiguousarray(
        cache_k.transpose(2, 0, 3, 1)).astype(f16)
    # V chunked per (h, b): [h, b, p, (n d)] with p = t within 128-chunk n
    cv = cache_v.reshape(B, NT, 128, NKV, HD)
    Vp_all = np.ascontiguousarray(
        cv.transpose(3, 0, 2, 1, 4)).reshape(NKV, B, 128, T).astype(f16)

    ones = np.ones((128, 128), f16)

    in_maps = []
    for h in range(N_CORES):
        wqkvT = np.concatenate([
            wq_r[h * NG * HD:(h + 1) * NG * HD],
            wk_r[h * HD:(h + 1) * HD],
            wv[h * HD:(h + 1) * HD],
        ], axis=0).T                                    # [4096, 768]
        # relayout [dk, c] -> [p, (chunk c)] with dk = 128*chunk + p
        wqkvT = np.ascontiguousarray(
            wqkvT.reshape(ND, 128, 768).transpose(1, 0, 2)
        ).reshape(128, ND * 768).astype(f16)
        woT = wo[:, h * NG * HD:(h + 1) * NG * HD].T    # [512, 4096]
        # relayout [dk, c] -> [p, (g c)] with dk = 128*g + p
        woT = np.ascontiguousarray(
            woT.reshape(NG, 128, DIM).transpose(1, 0, 2)
        ).reshape(128, NG * DIM).astype(f16)
        in_maps.append({
            "xTp": xTp,
            "wqkvT": wqkvT,
            "woT": woT,
            "KT": KT_all[h],
            "Vp": Vp_all[h],
            "ones": ones,
        })
    return in_maps


def _kernel_numpy_fallback(x, start_pos, freqs_cos, freqs_sin, cache_k, cache_v,
                           wq, wk, wv, wo):
    """Reference-equivalent numpy path for shapes this kernel isn't built for."""
    f32 = np.float32
    start_pos = int(start_pos)
    x = np.asarray(x, f32)
    bsz, seqlen, _ = x.shape
    n_rep = 4
    hd = HD

    def rope(t, c, s):
        tr = t.reshape(*t.shape[:-1], hd // 2, 2)
        a, b2 = tr[..., 0], tr[..., 1]
        c = c[None, :, None, :]
        s = s[None, :, None, :]
        out = np.stack([a * c - b2 * s, a * s + b2 * c], axis=-1)
        return out.reshape(t.shape)

    xq = (x @ np.asarray(wq, f32).T).reshape(bsz, seqlen, NKV * n_rep, hd)
    xk = (x @ np.asarray(wk, f32).T).reshape(bsz, seqlen, NKV, hd)
    xv = (x @ np.asarray(wv, f32).T).reshape(bsz, seqlen, NKV, hd)
    fc = np.asarray(freqs_cos, f32)
    fs = np.asarray(freqs_sin, f32)
    xq = rope(xq, fc, fs)
    xk = rope(xk, fc, fs)
    ck = np.array(cache_k, f32, copy=True)
    cvv = np.array(cache_v, f32, copy=True)
    ck[:, start_pos:start_pos + seqlen] = xk
    cvv[:, start_pos:start_pos + seqlen] = xv
    keys = ck[:, :start_pos + seqlen]
    values = cvv[:, :start_pos + seqlen]
    q = xq.reshape(bsz, seqlen, NKV, n_rep, hd)
    scale = 1.0 / np.sqrt(hd)
    scores = np.einsum('bsgrd,btgd->bgrst', q, keys) * scale
    scores = scores - scores.max(axis=-1, keepdims=True)
    e = np.exp(scores)
    probs = e / e.sum(axis=-1, keepdims=True)
    out = np.einsum('bgrst,btgd->bsgrd', probs, values)
    out = out.reshape(bsz, seqlen, NKV * n_rep * hd)
    return (out @ np.asarray(wo, f32).T).astype(f32)


TRACE = False          # set True (e.g. from test.py) to neuron-profile the run
TRACE_KWARGS = {}
LAST_RESULT = None     # BassKernelResults of the most recent device run


def kernel(x, start_pos, freqs_cos, freqs_sin, cache_k, cache_v, wq, wk, wv, wo):
    global LAST_RESULT
    x = np.asarray(x)
    if (int(start_pos) != T - 1 or x.shape != (B, 1, DIM)
            or np.asarray(cache_k).shape != (B, T, NKV, HD)):
        return _kernel_numpy_fallback(x, start_pos, freqs_cos, freqs_sin,
                                      cache_k, cache_v, wq, wk, wv, wo)

    from concourse.bass_utils import run_bass_kernel_spmd

    nc = _get_program()
    in_maps = _host_prep(x, freqs_cos, freqs_sin, cache_k, cache_v,
                         wq, wk, wv, wo)
    res = run_bass_kernel_spmd(nc, in_maps, list(range(N_CORES)),
                               trace=TRACE, **TRACE_KWARGS)
    LAST_RESULT = res
    out = np.zeros((B, DIM), np.float64)
    for i in range(N_CORES):
        # outT layout [p, (dchunk, b)] -> [B, DIM]
        o = res.results[i]["outT"].reshape(128, 32, B)
        out += o.transpose(2, 1, 0).reshape(B, DIM)
    return out.astype(np.float32).reshape(B, 1, DIM)


# revision 27
# speedup vs baseline: 1.0756x; 1.0756x over previous
"""GQA decode attention (B=32, q_len=1, T=4096, 32 q heads / 8 kv heads, hd=128)
on 8 Trainium2 NeuronCores.

Sharding: tensor-parallel over kv heads - core h owns kv head h (4 q heads),
its slice of wq/wk/wv (ColumnParallel) and wo (RowParallel), and the
cache_k/cache_v slices for that head. Each core computes a partial output
(RowParallel wo) in transposed layout; the host sums the 8 partials.

The kernel is HBM-bandwidth-bound (KV cache streaming), so everything is
fp16 end to end (rel err ~6e-4 vs the fp32 reference, measured on the
actual data):
  - q_len==1 means RoPE is a fixed linear map on the projection outputs, so
    it is folded into wq/wk on the host: w_rot = R(freqs) @ w. The
    1/sqrt(head_dim) score scale is folded into wq too.
  - K cache is stored transposed [hd, t] in fp16: one score matmul per
    128-key tile (K-tile stationary, fast-weight-load path; q streams 4
    columns).
  - V cache is stored [t, d] in fp16 and used stationary in the PV matmul
    (probs stream 4 columns), producing attn directly in [d, g] layout -
    no per-batch transpose.
  - softmax runs unnormalized (exp in fp32 PSUM -> fp16 probs); the
    denominator comes from a ones-column matmul (column sums) + a strided
    DVE reduce, and the normalization uses a ones-matmul broadcast of
    1/sum across partitions.
  - big DMA is split over three hardware rings: K on the sync ring, V on
    the scalar ring, weights/consts on the gpsimd ring, so the 16 DMA
    engines see deeper queues.
"""

import numpy as np

B = 32
DIM = 4096
HD = 128
NKV = 8
NG = 4          # q heads per kv head
T = 4096
NT = 32         # T / 128 key tiles
ND = 32         # DIM / 128 contraction chunks
N_CORES = 8

_PROG_CACHE = {}


def _build_program():
    import concourse.mybir as mybir
    import concourse.tile as tile
    from concourse import bacc

    fp32 = mybir.dt.float32
    f16 = mybir.dt.float16
    af = mybir.ActivationFunctionType
    ax = mybir.AxisListType
    alu = mybir.AluOpType

    nc = bacc.Bacc("TRN2", target_bir_lowering=False, debug=False,
                   num_devices=N_CORES)

    xTp_d = nc.dram_tensor("xTp", [128, ND * B], f16, kind="ExternalInput").ap()
    # wqkvT relayout [p, (piece chunk col)]: 4 pieces of 8 dk-chunks, so each
    # piece is one DMA with 8*768*2 = 12KB contiguous per partition
    wqkvT_d = nc.dram_tensor("wqkvT", [128, 4 * 8 * 768], f16,
                             kind="ExternalInput").ap()
    # woT relayout [p, (g col)]: one DMA, 32KB contiguous per partition
    woT_d = nc.dram_tensor("woT", [128, NG * DIM], f16, kind="ExternalInput").ap()
    KT_d = nc.dram_tensor("KT", [B, HD, T], f16, kind="ExternalInput").ap()
    Vp_d = nc.dram_tensor("Vp", [B, 128, T], f16, kind="ExternalInput").ap()
    ones_d = nc.dram_tensor("ones", [128, 128], f16, kind="ExternalInput").ap()
    # transposed partial output, layout [p, (dchunk, b)]
    out_d = nc.dram_tensor("outT", [128, 32 * B], f16, kind="ExternalOutput").ap()

    with tile.TileContext(nc) as tc:
        from contextlib import ExitStack
        with ExitStack() as ctx:
            const_pool = ctx.enter_context(tc.tile_pool(name="const", bufs=1))
            kv_pool = ctx.enter_context(tc.tile_pool(name="kv", bufs=7))
            small = ctx.enter_context(tc.tile_pool(name="small", bufs=3))

            # consts + weights on the gpsimd ring (keeps sync/scalar rings
            # free for the KV stream)
            xTp_sb = const_pool.tile([128, ND * B], f16, name="xTp_sb")
            nc.sync.dma_start(xTp_sb[:], xTp_d[:])
            ones_sb = const_pool.tile([128, 128], f16, name="ones_sb")
            nc.scalar.dma_start(ones_sb[:], ones_d[:])
            # woT tile: loaded in a single big-packet DMA, delayed behind the
            # projections (see the dummy-dep below) so the startup bandwidth
            # all goes to the wqkv weights that gate attention compute
            woT_sb = const_pool.tile([128, NG * DIM], f16, name="woT_sb")

            # ---- QKV projections: qT[o,b], kT[o,b], v[b,o] ----
            qT_sb = const_pool.tile([128, NG * B], f16, name="qT_sb")
            kT_sb = const_pool.tile([128, B], f16, name="kT_sb")
            v_sb = const_pool.tile([B, HD], f16, name="v_sb")

            # wqkv halves: two big-packet DMAs, one per fast ring, as const
            # tiles (no pool reuse -> no WAR dep -> they queue ahead of the
            # K/V stream in each ring's FIFO)
            wqh_sb = []
            for p, eng in enumerate([nc.sync, nc.scalar]):
                t = const_pool.tile([128, 16 * 768], f16, name=f"wqh{p}",
                                    tag=f"wqh{p}")
                eng.dma_start(t[:], wqkvT_d[:, 16 * 768 * p:16 * 768 * (p + 1)])
                wqh_sb.append(t)

            with tc.tile_pool(name="ppsum", bufs=1, space="PSUM") as ppsum:
                psq = [ppsum.tile([128, B], fp32, name=f"psq{g}", tag=f"psq{g}")
                       for g in range(NG)]
                psk = ppsum.tile([128, B], fp32, name="psk", tag="psk")
                psv = ppsum.tile([B, HD], fp32, name="psv", tag="psv")
                for n in range(ND):
                    wch = wqh_sb[n // 16][:, 768 * (n % 16):768 * (n % 16 + 1)]
                    xch = xTp_sb[:, B * n:B * (n + 1)]
                    st, sp = (n == 0), (n == ND - 1)
                    for g in range(NG):
                        nc.tensor.matmul(psq[g][:],
                                         wch[:, 128 * g:128 * (g + 1)],
                                         xch, start=st, stop=sp)
                    nc.tensor.matmul(psk[:], wch[:, 512:640], xch,
                                     start=st, stop=sp)
                    nc.tensor.matmul(psv[:], xch, wch[:, 640:768],
                                     start=st, stop=sp)
                for g in range(NG):
                    nc.vector.tensor_copy(qT_sb[:, B * g:B * (g + 1)], psq[g][:])
                nc.vector.tensor_copy(kT_sb[:], psk[:])
                nc.vector.tensor_copy(v_sb[:], psv[:])

            spsum = ctx.enter_context(tc.tile_pool(name="spsum", bufs=3, space="PSUM"))
            opsum = ctx.enter_context(tc.tile_pool(name="opsum", bufs=3, space="PSUM"))
            wpsum = ctx.enter_context(tc.tile_pool(name="wpsum", bufs=2, space="PSUM"))

            qT_re = qT_sb.rearrange("p (g b) -> p b g", b=B)
            attnT_sb = const_pool.tile([128, NG * B], f16, name="attnT_sb")
            attnT_re = attnT_sb.rearrange("p (g b) -> p b g", b=B)

            outT_sb = const_pool.tile([128, 32 * B], f16, name="outT_sb")

            def wo_part(h0, nb):
                # out[dout, b] for batches [h0, h0+nb): woT-stationary
                for j in range(32):
                    psW = wpsum.tile([128, nb], fp32, name="psW", tag="psW")
                    for g in range(NG):
                        nc.tensor.matmul(
                            psW[:],
                            woT_sb[:, DIM * g + 128 * j:DIM * g + 128 * (j + 1)],
                            attnT_sb[:, B * g + h0:B * g + h0 + nb],
                            start=(g == 0), stop=(g == NG - 1))
                    nc.vector.tensor_copy(
                        outT_sb[:, B * j + h0:B * j + h0 + nb], psW[:])

            # ---- attention, software-pipelined (depth 2) ----
            # iteration i emits: loads+scores+exp for batch i, PV+colsum+
            # denominator chain for batch i-1, broadcast+normalize for batch
            # i-2 -- so every tensor-queue op has its deps resolved a full
            # batch ahead and the in-order tensor engine never stalls on the
            # DVE round-trip.
            Vt = {}     # per-batch live tiles
            Pt = {}
            Ot = {}
            RRt = {}

            def stage_load_scores(b):
                # weighted ring split ~5:3 toward the sync ring
                ek = nc.scalar if b % 4 == 2 else nc.sync
                ev = nc.sync if b % 4 in (1, 2) else nc.scalar
                K_sb = kv_pool.tile([128, T], f16, name="K_sb", tag="K")
                ek.dma_start(K_sb[:], KT_d[b])
                V_sb = kv_pool.tile([128, T], f16, name="V_sb", tag="V")
                ev.dma_start(V_sb[:], Vp_d[b])
                Vt[b] = V_sb
                # new-token key: overwrite cache column t=4095
                nc.vector.tensor_copy(K_sb[:, T - 1:T], kT_sb[:, b:b + 1])
                # new-token value: overwrite the t=4095 V row (partition 127
                # of the last chunk). Cross-partition move -> tiny DMA.
                nc.gpsimd.dma_start(
                    V_sb[127:128, 128 * (NT - 1):128 * NT],
                    v_sb[b:b + 1, 0:HD])
                qb = qT_re[:, b]  # [128, 4] strided
                psS = spsum.tile([128, NG * NT], fp32, name="psS", tag="psS")
                for n in range(NT):
                    nc.tensor.matmul(psS[:, NG * n:NG * (n + 1)],
                                     K_sb[:, 128 * n:128 * (n + 1)], qb,
                                     start=True, stop=True)
                probs = kv_pool.tile([128, NG * NT], f16, name="probs",
                                     tag="probs")
                for c in range(2):
                    cw = NG * NT // 2
                    nc.scalar.activation(probs[:, cw * c:cw * (c + 1)],
                                         psS[:, cw * c:cw * (c + 1)], af.Exp)
                Pt[b] = probs

            def stage_pv_denom(b):
                V_sb, probs = Vt.pop(b), Pt[b]
                # one PSUM bank: cols [0,4) = PV out [d, g]; cols [4,8) =
                # broadcast 1/sum; cols [8,136) partition 0 = column sums
                psO = opsum.tile([128, 8 + NG * NT], fp32, name="psO",
                                 tag="psO")
                for n in range(NT):
                    nc.tensor.matmul(psO[:, 0:NG],
                                     V_sb[:, 128 * n:128 * (n + 1)],
                                     probs[:, NG * n:NG * (n + 1)],
                                     start=(n == 0), stop=(n == NT - 1))
                nc.tensor.matmul(psO[0:1, 8:8 + NG * NT], ones_sb[:, 0:1],
                                 probs[:], start=True, stop=True)
                Ot[b] = psO
                sums4 = small.tile([1, NG], fp32, name="sums4", tag="sums4")
                nc.vector.tensor_reduce(
                    sums4[:],
                    psO[0:1, 8:8 + NG * NT].rearrange("p (n g) -> p g n", g=NG),
                    axis=ax.X, op=alu.add)
                recip = small.tile([1, NG], fp32, name="recip", tag="recip")
                nc.vector.reciprocal(recip[:], sums4[:])
                rr = small.tile([128, NG], f16, name="rr", tag="rr")
                nc.vector.memset(rr[:], 0.0)
                nc.vector.tensor_copy(rr[0:1, :], recip[:])
                RRt[b] = rr

            def stage_normalize(b):
                psO, rr = Ot.pop(b), RRt.pop(b)
                del Pt[b]
                nc.tensor.matmul(psO[:, NG:2 * NG], ones_sb[:], rr[:],
                                 start=True, stop=True)
                bc_sb = small.tile([128, NG], fp32, name="bc_sb", tag="bc_sb")
                nc.vector.tensor_copy(bc_sb[:], psO[:, NG:2 * NG])
                nc.vector.tensor_mul(attnT_re[:, b], psO[:, 0:NG], bc_sb[:])

            for i in range(B + 2):
                if i == 24:
                    # woT in four 1MB chunks, each held behind a later attnT
                    # column, so the load trickles into the late-stream slack
                    # of the otherwise-idle gpsimd ring instead of one burst
                    for g in range(NG):
                        col = 10 + 4 * g
                        nc.vector.tensor_copy(
                            woT_sb[0:1, DIM * g:DIM * g + 1],
                            attnT_sb[0:1, col:col + 1])
                        nc.gpsimd.dma_start(
                            woT_sb[:, DIM * g:DIM * (g + 1)],
                            woT_d[:, DIM * g:DIM * (g + 1)])
                if i == 28:
                    # wo for batches 0..15: tensor-idle-gap work while the
                    # last batches stream (needs woT + attnT[0:16])
                    wo_part(0, 16)
                if i >= 2:
                    stage_normalize(i - 2)
                if 1 <= i <= B:
                    stage_pv_denom(i - 1)
                if i < B:
                    stage_load_scores(i)

            # ---- second wo half (batches 16..31) + output store ----
            wo_part(B // 2, B // 2)
            nc.sync.dma_start(out_d[:], outT_sb[:])

    nc.compile()
    return nc


def _get_program():
    if "nc" not in _PROG_CACHE:
        _PROG_CACHE["nc"] = _build_program()
    return _PROG_CACHE["nc"]


def _host_prep(x, freqs_cos, freqs_sin, cache_k, cache_v, wq, wk, wv, wo):
    """Build the 8 per-core input maps (all fp16)."""
    f32 = np.float32
    f16 = np.float16
    x = np.asarray(x, f32)
    cos = np.asarray(freqs_cos, f32).reshape(-1)[:HD // 2]
    sin = np.asarray(freqs_sin, f32).reshape(-1)[:HD // 2]
    wq = np.asarray(wq, f32)
    wk = np.asarray(wk, f32)
    wv = np.asarray(wv, f32)
    wo = np.asarray(wo, f32)
    cache_k = np.asarray(cache_k, f32)
    cache_v = np.asarray(cache_v, f32)

    def rope_fold(w, nheads):
        w4 = w.reshape(nheads, HD // 2, 2, DIM)
        a, bb = w4[:, :, 0, :], w4[:, :, 1, :]
        c = cos[None, :, None]
        s = sin[None, :, None]
        out = np.empty_like(w4)
        out[:, :, 0, :] = a * c - bb * s
        out[:, :, 1, :] = a * s + bb * c
        return out.reshape(nheads * HD, DIM)

    wq_r = rope_fold(wq, NKV * NG) * f32(1.0 / np.sqrt(HD))
    wk_r = rope_fold(wk, NKV)

    x2 = x.reshape(B, DIM)
    xTp = np.ascontiguousarray(
        x2.T.reshape(ND, 128, B).transpose(1, 0, 2)).reshape(128, ND * B)
    xTp = xTp.astype(f16)

    # K transposed per (h, b): [h, b, d, t]
    KT_all = np.ascontiguousarray(
        cache_k.transpose(2, 0, 3, 1)).astype(f16)
    # V chunked per (h, b): [h, b, p, (n d)] with p = t within 128-chunk n
    cv = cache_v.reshape(B, NT, 128, NKV, HD)
    Vp_all = np.ascontiguousarray(
        cv.transpose(3, 0, 2, 1, 4)).reshape(NKV, B, 128, T).astype(f16)

    ones = np.ones((128, 128), f16)

    in_maps = []
    for h in range(N_CORES):
        wqkvT = np.concatenate([
            wq_r[h * NG * HD:(h + 1) * NG * HD],
            wk_r[h * HD:(h + 1) * HD],
            wv[h * HD:(h + 1) * HD],
        ], axis=0).T                                    # [4096, 768]
        # relayout [dk, c] -> [p, (chunk c)] with dk = 128*chunk + p
        wqkvT = np.ascontiguousarray(
            wqkvT.reshape(ND, 128, 768).transpose(1, 0, 2)
        ).reshape(128, ND * 768).astype(f16)
        woT = wo[:, h * NG * HD:(h + 1) * NG * HD].T    # [512, 4096]
        # relayout [dk, c] -> [p, (g c)] with dk = 128*g + p
        woT = np.ascontiguousarray(
            woT.reshape(NG, 128, DIM).transpose(1, 0, 2)
        ).reshape(128, NG * DIM).astype(f16)
        in_maps.append({
            "xTp": xTp,
            "wqkvT": wqkvT,
            "woT": woT,
            "KT": KT_all[h],
            "Vp": Vp_all[h],
            "ones": ones,
        })
    return in_maps


def _kernel_numpy_fallback(x, start_pos, freqs_cos, freqs_sin, cache_k, cache_v,
                           wq, wk, wv, wo):
    """Reference-equivalent numpy path for shapes this kernel isn't built for."""
    f32 = np.float32
    start_pos = int(start_pos)
    x = np.asarray(x, f32)
    bsz, seqlen, _ = x.shape
    n_rep = 4
    hd = HD

    def rope(t, c, s):
        tr = t.reshape(*t.shape[:-1], hd // 2, 2)
        a, b2 = tr[..., 0], tr[..., 1]
        c = c[None, :, None, :]
        s = s[None, :, None, :]
        out = np.stack([a * c - b2 * s, a * s + b2 * c], axis=-1)
        return out.reshape(t.shape)

    xq = (x @ np.asarray(wq, f32).T).reshape(bsz, seqlen, NKV * n_rep, hd)
    xk = (x @ np.asarray(wk, f32).T).reshape(bsz, seqlen, NKV, hd)
    xv = (x @ np.asarray(wv, f32).T).reshape(bsz, seqlen, NKV, hd)
    fc = np.asarray(freqs_cos, f32)
    fs = np.asarray(freqs_sin, f32)
    xq = rope(xq, fc, fs)
    xk = rope(xk, fc, fs)
    ck = np.array(cache_k, f32, copy=True)
    cvv = np.array(cache_v, f32, copy=True)
    ck[:, start_pos:start_pos + seqlen] = xk
    cvv[:, start_pos:start_pos + seqlen] = xv
    keys = ck[:, :start_pos + seqlen]
    values = cvv[:, :start_pos + seqlen]
    q = xq.reshape(bsz, seqlen, NKV, n_rep, hd)
    scale = 1.0 / np.sqrt(hd)
    scores = np.einsum('bsgrd,btgd->bgrst', q, keys) * scale
    scores = scores - scores.max(axis=-1, keepdims=True)
    e = np.exp(scores)
    probs = e / e.sum(axis=-1, keepdims=True)
    out = np.einsum('bgrst,btgd->bsgrd', probs, values)
    out = out.reshape(bsz, seqlen, NKV * n_rep * hd)
    return (out @ np.asarray(wo, f32).T).astype(f32)


TRACE = False          # set True (e.g. from test.py) to neuron-profile the run
TRACE_KWARGS = {}
LAST_RESULT = None     # BassKernelResults of the most recent device run


def kernel(x, start_pos, freqs_cos, freqs_sin, cache_k, cache_v, wq, wk, wv, wo):
    global LAST_RESULT
    x = np.asarray(x)
    if (int(start_pos) != T - 1 or x.shape != (B, 1, DIM)
            or np.asarray(cache_k).shape != (B, T, NKV, HD)):
        return _kernel_numpy_fallback(x, start_pos, freqs_cos, freqs_sin,
                                      cache_k, cache_v, wq, wk, wv, wo)

    from concourse.bass_utils import run_bass_kernel_spmd

    nc = _get_program()
    in_maps = _host_prep(x, freqs_cos, freqs_sin, cache_k, cache_v,
                         wq, wk, wv, wo)
    res = run_bass_kernel_spmd(nc, in_maps, list(range(N_CORES)),
                               trace=TRACE, **TRACE_KWARGS)
    LAST_RESULT = res
    out = np.zeros((B, DIM), np.float64)
    for i in range(N_CORES):
        # outT layout [p, (dchunk, b)] -> [B, DIM]
        o = res.results[i]["outT"].reshape(128, 32, B)
        out += o.transpose(2, 1, 0).reshape(B, DIM)
    return out.astype(np.float32).reshape(B, 1, DIM)
